# revision 52
# baseline (speedup 1.0000x reference)
"""GQA attention block (RoPE + causal softmax + out-projection) on 8 TRN2 cores.

Problem: q (2, 2048, 1024) 16 heads, k/v (2, 2048, 256) 4 kv heads (GQA rep 4),
causal attention, out @ w_out (1024, 1024).

Sharding: core c = (batch b = c//4, kv group = c%4). Each core computes its 4
q-heads x full T attention against its kv head, then the partial projection
X_heads @ w_out[head_rows, :]; the host sums the 4 partials per batch.

Layout: everything computed transposed (channels on partitions, sequence on
the free axis). Per 512-wide q group, the 4 q heads are processed as two
sequential PAIRS (even head at partitions 0-63, odd at 64-127):
  - S^T for a pair lives in one [128, 1024] psum tile (2 banks, 2-buf).
    exp(S/8 - 2.5) is one ACT instruction per round (ACT is the pacing
    engine, ~78us of exp); the -2.5 bias keeps fp8 P under 240 and cancels
    in the softmax division.
  - SUB-DIAGONAL rounds: exp emits fp8e4 P^T into [128, 2, 1024] pair
    tiles; O^T accumulates via fp8 DoubleRow matmuls (2 k-blocks per pass,
    0.5 cyc/row; vaug8 padded to 80 cols for the LDWEIGHTS stride%16 rule).
    DIAGONAL rounds stay bf16 (regular matmuls) so the dominant
    near-diagonal weights keep precision: rel err 2.9e-3 vs 2.2e-2 all-fp8.
  - O matmuls are EMITTED 2 rounds late (pend/flush): the in-order PE FIFO
    otherwise head-of-line blocks S(kb+1) behind exp(kb).
  - causal masking: diagonal 128x128 strip masked MULTIPLICATIVELY on bf16
    P^T after exp (psum-accumulate masks silently break on HW).
  - O^T accumulates [V | 1] stationary; psum row 64 is the denominator.
    Normalize per head: den copy (DVE; ACT in the tail), 
    reciprocal_approx_fast (SBUF-only input), GpSimd partition_broadcast,
    one multiply.
  - projection (chunk n: w[:,n*128:+128]^T @ X^T) is DEFERRED into the next
    group's pair-A rounds (psum borrows the idle pair-B oT banks); group
    3's projection is split per channel-half into d_out3 partials so half
    streams out during pair B. Output DMAs alternate the SP/GpSimd rings.
  - RoPE per 512-col chunk, interleaved so each group's q/k are ready well
    before its first S matmul; independent muls emitted before
    psum-dependent ops (the DVE chain is the startup critical path).
  - input DMAs: SDMA engines drain descriptors in issue order and pay a
    fixed cost per descriptor, so chunk-0 and chunk-1 rope inputs are
    host-packed into two [128, ~2.7K] "head" slabs (one DMA, 128
    descriptors each) issued before the bulk; remaining loads are
    whole-tensor, need-ordered across the SP/ACT/GpSimd rings.
  - HAM: warmup burst + keep-warm fillers in the sparse early groups hold
    the PE at 2.4 GHz.
rotate_half is a signed-permutation matmul; sin/cos tables are host-built
[64, T] (rows repeat with period 32, k-heads use rows 0-63).
"""

import sys

if "/opt/trn_rl_repo" not in sys.path:
    sys.path.insert(0, "/opt/trn_rl_repo")

import numpy as np

B, T, D, NH, NKV, HD = 2, 2048, 1024, 16, 4, 64
HC = NH // NKV          # q heads per core = 4
CD = HC * HD            # per-core channel dim = 256
KVD = HD                # per-core kv channel dim = 64
NCORES = 8
QB = 128                # q/k block
NG = T // 512           # q column groups of 512
NKB = T // QB           # k blocks = 16
MASK = -240.0           # pre-scale additive mask; exp(-240/8) = exp(-30) ~ 1e-13
EBIAS = -2.5            # exp bias shift: keeps fp8 P in [~0, 104] < 240 (fp8e4
                        # overflows to Inf above 240); cancels in softmax

_cache: dict = {}


def _tables():
    if "tables" in _cache:
        return _cache["tables"]
    p = np.arange(64)
    t = np.arange(T)
    ang = t[None, :] / (10000.0 ** ((p[:, None] % 32) / 32.0))
    cosT = np.cos(ang).astype(np.float32)   # [64, T]
    sinT = np.sin(ang).astype(np.float32)

    rotP = np.zeros((128, 128), np.float32)
    for base in (0, 64):
        for i in range(32):
            rotP[base + 32 + i, base + i] = -1.0   # out[i] = -x[i+32]
            rotP[base + i, base + 32 + i] = 1.0    # out[i+32] = x[i]

    kk = np.arange(QB)
    # multiplicative causal mask for the diagonal 128x128 strip: P *= mask01
    mask01 = np.where(kk[:, None] <= kk[None, :], 1.0, 0.0).astype(np.float32)
    _cache["tables"] = (cosT, sinT, rotP, mask01)
    return _cache["tables"]


def _build():
    import concourse.tile as tile
    from concourse import bacc, mybir

    f32 = mybir.dt.float32
    bf16 = mybir.dt.bfloat16
    f8 = mybir.dt.float8e4
    Exp = mybir.ActivationFunctionType.Exp
    DR = mybir.MatmulPerfMode.DoubleRow

    nc = bacc.Bacc("TRN2", target_bir_lowering=False, debug=False,
                   num_devices=NCORES)

    d_qT = nc.dram_tensor("qT", [CD, T], bf16, kind="ExternalInput")
    d_kT = nc.dram_tensor("kT", [KVD, T], bf16, kind="ExternalInput")
    d_vaug = nc.dram_tensor("vaug", [128, NKB * (HD + 1)], bf16,
                        kind="ExternalInput")
    # fp8 V padded to 80 cols: DoubleRow LDWEIGHTS requires the k-subtile
    # stride to be a multiple of 16 (s3_lw_dual_fp8_restrictions). Both V
    # slabs are host-pre-arranged to the SBUF layout so the input DMA is one
    # contiguous per-partition strip (a (n p) m gather costs 1-6us of HWDGE
    # descriptor-issue time on the ring).
    d_vaug8 = nc.dram_tensor("vaug8", [128, NKB * 80], f8,
                             kind="ExternalInput")
    d_w = nc.dram_tensor("w", [CD, D], bf16, kind="ExternalInput")
    d_cosT = nc.dram_tensor("cosT", [64, T], bf16, kind="ExternalInput")
    d_sinT = nc.dram_tensor("sinT", [64, T], bf16, kind="ExternalInput")
    # packed "head" slab: everything rope chunk 0 needs in ONE DMA (SDMA
    # pays ~0.4us per 8-descriptor engine queue — six separate head loads
    # cost ~500 descriptors and land at 12-15us; one slab = 128, lands ~10)
    # layout per partition: [q0c0 512 | q1c0 512 | rotP 128 | mask 128 |
    #                        kc0 512 (lo only) | cos_c0 512 | sin_c0 512]
    d_head = nc.dram_tensor("head", [128, 2816], bf16, kind="ExternalInput")
    # chunk-1 slab: [q0c1 512 | q1c1 512 | kc1 512 (lo) | cos_c1 | sin_c1]
    d_head2 = nc.dram_tensor("head2", [128, 2560], bf16, kind="ExternalInput")
    d_outT = nc.dram_tensor("outT", [D, T], bf16, kind="ExternalOutput")
    # group-3 projection partials (w0@x0 and w1@x1 halves, host-summed) so
    # half the tail projection can stream out during pair B
    d_out3 = nc.dram_tensor("out3", [2, D, 512], bf16, kind="ExternalOutput")

    with tile.TileContext(nc) as tc:
        with (
            tc.tile_pool(name="consts", bufs=1) as consts,
            tc.tile_pool(name="data", bufs=1) as data,
            tc.tile_pool(name="pt", bufs=8) as ptp,
            tc.tile_pool(name="small", bufs=6) as small,
            tc.tile_pool(name="psS", bufs=2, space="PSUM") as psS,
            tc.tile_pool(name="psO", bufs=1, space="PSUM") as psO,
        ):
            cosT = consts.tile([128, T], bf16)
            sinT = consts.tile([128, T], bf16)
            head = consts.tile([128, 2816], bf16)
            HQ0, HQ1, HRP, HMK, HKC, HCS, HSN = (0, 512, 1024, 1152, 1280,
                                                 1792, 2304)
            head2 = consts.tile([128, 2560], bf16)
            JQ0, JQ1, JKC, JCS, JSN = 0, 512, 1024, 1536, 2048
            ebias = consts.tile([128, 1], f32)
            nc.vector.memset(ebias[:], EBIAS)
            nc.sync.dma_start(head[:], d_head[:])

            qT = [data.tile([128, T], bf16, name=f"qT{i}", tag=f"qT{i}")
                  for i in range(2)]
            kT = data.tile([KVD, T], bf16, tag="kT")
            kThi = data.tile([128, T], bf16, tag="kThi")
            vaug = data.tile([128, NKB, HD + 1], bf16, tag="vaug")
            vaug8 = data.tile([128, NKB, 80], f8, tag="vaug8")
            w = [data.tile([128, D], bf16, name=f"w{i}", tag=f"w{i}")
                 for i in range(2)]
            xT = [data.tile([128, T], bf16, name=f"xT{i}", tag=f"xT{i}")
                  for i in range(2)]

            # tiny exp reading the locally-memset tile (no DMA dep) so
            # walrus's ACT_TABLE_LOAD runs immediately, before the ACT-ring
            # input DMAs occupy the queue
            dum = data.tile([128, 512], bf16, tag="dum")
            nc.vector.memset(dum[:], 1.0)
            # ones column for the tail's PE-side reciprocal broadcast
            onesc = data.tile([1, 64], f32, tag="onesc")
            nc.vector.memset(onesc[:], 1.0)
            dummy = small.tile([1, 8], bf16, tag="dummy")
            nc.scalar.activation(dummy[:], dum[0:1, 0:8], Exp, scale=0.125)

            # PE warmup: DMA-independent dense burst so HAM reaches full
            # clock early; 16 x N=512 cold matmuls (~7us) guarantee covering
            # a full free-running HAM activity window
            for _ in range(5):
                warm = psS.tile([128, 1024], f32, name="warm", tag="S")
                for j in range(2):
                    nc.tensor.matmul(warm[:, j * 512:(j + 1) * 512],
                                     dum[:, 0:128], dum[:],
                                     start=True, stop=True)

            # whole-tensor input DMAs. HWDGE issue is serial per ring
            # (~650ns each), so the rope-chunk-0 deps are split across BOTH
            # HWDGE rings (SP + ACT — ACT is idle until the first exp at
            # ~13us anyway); bulky contiguous loads ride the GpSimd SWDGE
            # ring. Ring order = first-use order.
            # Input DMA order = SDMA drain order = need order. After the
            # head slab: tails (cols 512:T) of q/k/cos/sin (rope chunk 1 at
            # ~12.5us needs their first third), then late consumers.
            tl = slice(1024, T)
            nc.sync.dma_start(head2[:], d_head2[:])
            nc.sync.dma_start(qT[0][:, tl], d_qT[0:128, tl])
            nc.sync.dma_start(kT[:, tl], d_kT[:, tl])
            nc.sync.dma_start(cosT[0:64, tl], d_cosT[:, tl])
            nc.sync.dma_start(sinT[0:64, tl], d_sinT[:, tl])
            nc.scalar.dma_start(cosT[64:128, tl], d_cosT[:, tl])
            nc.scalar.dma_start(sinT[64:128, tl], d_sinT[:, tl])
            nc.sync.dma_start(qT[1][:, tl], d_qT[128:256, tl])
            nc.gpsimd.dma_start(
                vaug[:].rearrange("p n m -> p (n m)"), d_vaug[:])
            nc.gpsimd.dma_start(
                vaug8[:].rearrange("p n m -> p (n m)"), d_vaug8[:])
            nc.gpsimd.dma_start(w[0][:], d_w[0:128, :])
            nc.gpsimd.dma_start(w[1][:], d_w[128:256, :])

            def rope_chunk_a(f, tq="oT0", tk="oT1"):
                # psum scratch borrows idle oT banks (oT2/3 during pair A
                # after the proj chunks; oT0/1 during prologue/pair B); the S
                # pool is never touched so the exp pipeline keeps its 2 slots.
                # Part a: qT[0] + kT (all the next pair A needs).
                sl = slice(f * 512, (f + 1) * 512)
                if f == 0:
                    qs = head[:, HQ0:HQ0 + 512]
                    ks = head[0:64, HKC:HKC + 512]
                    co, si = head[:, HCS:HCS + 512], head[:, HSN:HSN + 512]
                    co64 = head[0:64, HCS:HCS + 512]
                    si64 = head[0:64, HSN:HSN + 512]
                elif f == 1:
                    qs = head2[:, JQ0:JQ0 + 512]
                    ks = head2[0:64, JKC:JKC + 512]
                    co, si = head2[:, JCS:JCS + 512], head2[:, JSN:JSN + 512]
                    co64 = head2[0:64, JCS:JCS + 512]
                    si64 = head2[0:64, JSN:JSN + 512]
                else:
                    qs, ks = qT[0][:, sl], kT[:, sl]
                    co, si = cosT[:, sl], sinT[:, sl]
                    co64, si64 = cosT[:KVD, sl], sinT[:KVD, sl]
                rk = psO.tile([128, 512], f32, name="rk", tag=tk)
                nc.tensor.matmul(rk[:KVD, :], head[0:64, HRP:HRP + 64],
                                 ks, start=True, stop=True)
                rq0 = psO.tile([128, 512], f32, name="rq0", tag=tq)
                nc.tensor.matmul(rq0[:], head[:, HRP:HRP + 128], qs,
                                 start=True, stop=True)
                # independent muls first, psum-dependent ops after — the DVE
                # chain is the startup critical path
                nc.vector.tensor_mul(kT[:, sl], ks, co64)
                nc.vector.tensor_mul(qT[0][:, sl], qs, co)
                nc.vector.tensor_mul(rk[:KVD, :], rk[:KVD, :], si64)
                nc.vector.tensor_add(kT[:, sl], kT[:, sl], rk[:KVD, :])
                nc.vector.tensor_copy(kThi[64:128, sl], kT[:, sl])
                nc.vector.tensor_mul(rq0[:], rq0[:], si)
                nc.vector.tensor_add(qT[0][:, sl], qT[0][:, sl], rq0[:])

            def rope_chunk_b(f, t="oT0"):
                # Part b: qT[1]; emitted a few rounds after part a so the
                # rq1 matmul's WAR wait on the rq0 slot is already satisfied
                # (an unsatisfied wait here would stall the whole PE FIFO).
                sl = slice(f * 512, (f + 1) * 512)
                if f == 0:
                    qs = head[:, HQ1:HQ1 + 512]
                    co, si = head[:, HCS:HCS + 512], head[:, HSN:HSN + 512]
                elif f == 1:
                    qs = head2[:, JQ1:JQ1 + 512]
                    co, si = head2[:, JCS:JCS + 512], head2[:, JSN:JSN + 512]
                else:
                    qs = qT[1][:, sl]
                    co, si = cosT[:, sl], sinT[:, sl]
                rq1 = psO.tile([128, 512], f32, name="rq1", tag=t)
                nc.tensor.matmul(rq1[:], head[:, HRP:HRP + 128], qs,
                                 start=True, stop=True)
                nc.vector.tensor_mul(rq1[:], rq1[:], si)
                nc.vector.tensor_mul(qT[1][:, sl], qs, co)
                nc.vector.tensor_add(qT[1][:, sl], qT[1][:, sl], rq1[:])

            def proj_chunk(gq, j, slot):
                # out^T[n-chunk j, qlo:qlo+512] = sum_cc w[cc]^T @ xT[cc];
                # output DMAs alternate the SP and GpSimd rings
                qlo = gq * 512
                pr = psO.tile([128, 512], f32, name="pr", tag=f"oT{slot}")
                for cc in range(2):
                    nc.tensor.matmul(pr[:], w[cc][:, j * 128:(j + 1) * 128],
                                     xT[cc][:, qlo:qlo + 512],
                                     start=(cc == 0), stop=(cc == 1))
                st = ptp.tile([128, 512], bf16, name="st", tag="st")
                nc.vector.tensor_copy(st[:], pr[:])
                eng = nc.sync if j % 2 == 0 else nc.gpsimd
                eng.dma_start(d_outT[j * 128:(j + 1) * 128, qlo:qlo + 512],
                              st[:])

            def proj3_half(cc, j, slot, st_eng, dma_eng):
                # group-3 projection, single-cc half: streams to d_out3[cc]
                pr = psO.tile([128, 512], f32, name="pr3", tag=f"oT{slot}")
                nc.tensor.matmul(pr[:], w[cc][:, j * 128:(j + 1) * 128],
                                 xT[cc][:, 3 * 512:4 * 512],
                                 start=True, stop=True)
                st = ptp.tile([128, 512], bf16, name="st3", tag="st")
                st_eng(st[:], pr[:])
                dma_eng(d_out3[cc, j * 128:(j + 1) * 128, :], st[:])

            def attn_group(g):
                qlo = g * 512
                nkb = (qlo + 512) // QB
                kb0 = qlo // QB
                for pair in range(2):
                    qt = qT[pair]
                    xt = xT[pair]
                    oT = [psO.tile([HD + 1, 512], f32, name=f"oT{2*pair+j}",
                                   tag=f"oT{2*pair+j}") for j in range(2)]
                    PTd = None
                    # O matmuls are EMITTED 2 rounds late: in-order PE FIFO
                    # means an O placed right after its exp head-of-line
                    # blocks S(kb+1) behind exp(kb) — round time becomes
                    # S+exp+O serial instead of exp-paced. Two rounds of
                    # slack (the S-pool WAR already keeps PE <=2 rounds
                    # ahead) guarantees the O's inputs are long done.
                    pend = []

                    def flush_pend(upto_kb):
                        while pend and pend[0][0] <= upto_kb:
                            pend.pop(0)[1]()
                    for kb in range(nkb):
                        diag = kb >= kb0
                        cs = QB * (kb - kb0) if diag else 0
                        S2 = psS.tile([128, 1024], f32, name="S2", tag="S")
                        S2v = S2.rearrange("p (j n) -> p j n", j=2)
                        if g <= 1:
                            # keep-warm filler: early groups have too little
                            # PE work per round and HAM re-throttles the PE
                            # clock to 1.2 GHz without it
                            nc.tensor.matmul(S2[:, 0:128], dum[:, 0:128],
                                             dum[:, 0:128],
                                             start=True, stop=True)
                        for j in range(2):
                            qoff = j * 64
                            kTh = kT if j == 0 else kThi[64:128, :]
                            base = j * 512
                            nc.tensor.matmul(
                                S2[:, base + cs:base + 512],
                                kTh[:, kb * QB:(kb + 1) * QB],
                                qt[qoff:qoff + 64, qlo + cs:qlo + 512],
                                start=True, stop=True)
                        flush_pend(kb - 3)
                        if not diag:
                            # sub-diagonal rounds: exp straight to fp8 and
                            # accumulate O via DoubleRow (2 k-blocks per pass,
                            # 0.5 cyc/row) — diagonal strip stays bf16 so the
                            # dominant near-diagonal weights keep precision
                            if kb % 2 == 0:
                                PTd = ptp.tile([128, 2, 1024], f8,
                                               name="PTd", tag="PTd")
                            nc.scalar.activation(PTd[:, kb % 2, :], S2[:],
                                                 Exp, scale=0.125, bias=ebias[:])
                            if kb % 2 == 1:
                                def odr(kb=kb, PTd=PTd):
                                    for j in range(2):
                                        nc.tensor.matmul(
                                            oT[j][:, :],
                                            vaug8[:, kb - 1:kb + 1, 0:HD + 1],
                                            PTd[:, :, j * 512:(j + 1) * 512],
                                            start=(kb == 1), stop=False,
                                            perf_mode=DR,
                                            skip_group_check=True)
                                pend.append((kb, odr))
                        else:
                            PT = ptp.tile([128, 1024], bf16, name="PT",
                                          tag="PT")
                            if cs:
                                PTv = PT.rearrange("p (j n) -> p j n", j=2)
                                nc.scalar.activation(PTv[:, :, cs:],
                                                     S2v[:, :, cs:],
                                                     Exp, scale=0.125,
                                                     bias=ebias[:])
                            else:
                                nc.scalar.activation(PT[:], S2[:], Exp,
                                                     scale=0.125, bias=ebias[:])
                            # multiplicative causal mask on the diagonal
                            # 128-wide strip of each head's P^T (no psum
                            # has_written reliance — robust on HW)
                            for j in range(2):
                                nc.vector.tensor_mul(
                                    PT[:, j * 512 + cs:j * 512 + cs + QB],
                                    PT[:, j * 512 + cs:j * 512 + cs + QB],
                                    head[:, HMK:HMK + QB])

                            def odiag(kb=kb, cs=cs, PT=PT):
                                for j in range(2):
                                    nc.tensor.matmul(
                                        oT[j][:, cs:], vaug[:, kb, :],
                                        PT[:, j * 512 + cs:(j + 1) * 512],
                                        start=(kb == 0), stop=(kb == nkb - 1),
                                        skip_group_check=True)
                            pend.append((kb, odiag))
                        # deferred work hooks. For g1/g2 the projection
                        # burst is split across BOTH pairs (4 chunks each) —
                        # 8 chunks in pair A's first rounds starve ACT for
                        # ~4.6us; pair B's kb2/kb4 use the freed oT0/1 slots
                        if pair == 0 and 1 <= g <= 2 and 1 <= kb <= 4:
                            proj_chunk(g - 1, kb - 1, 2 + (kb % 2))
                        if pair == 0 and g == 3 and 1 <= kb <= 8:
                            proj_chunk(g - 1, kb - 1, 2 + (kb % 2))
                        if pair == 1 and 1 <= g <= 2 and kb in (2, 4):
                            for i in range(2):
                                proj_chunk(g - 1, 4 + (kb // 2 - 1) * 2 + i, i)
                        if pair == 0 and 1 <= g <= 2 and kb == 6:
                            # rope the next chunk early (pair A, borrowing
                            # the proj banks) so the DVE backlog of pair B
                            # can't delay the next group's q/k readiness
                            rope_chunk_a(g + 1, tq="oT2", tk="oT3")
                        if pair == 0 and g == 0 and kb == 0:
                            # q1 chunk 0 rides the head slab (lands ~10us);
                            # roping it here keeps the prologue FIFOs clear
                            rope_chunk_b(0, t="oT2")
                        if pair == 0 and g == 0 and kb == 1:
                            rope_chunk_a(1, tq="oT2", tk="oT3")
                        if pair == 0 and g == 0 and kb == 3:
                            rope_chunk_b(1, t="oT2")
                        if pair == 1 and kb == min(3, nkb - 2) and 1 <= g <= 2:
                            rope_chunk_b(g + 1)
                        if pair == 1 and g == 3 and 4 <= kb <= 11:
                            # stream group-3's w0@x0 projection half during
                            # pair B (oT0/oT1 banks are free: no rope here)
                            j3 = kb - 4
                            proj3_half(0, j3, j3 % 2,
                                       nc.vector.tensor_copy,
                                       nc.gpsimd.dma_start)
                    flush_pend(nkb)
                    # normalize the pair; reciprocal_approx_fast needs SBUF
                    # input (PSUM source diverges on HW), so stage the
                    # denominator row first. In the tail (last pair) the
                    # broadcast runs on the idle PE (ones-column matmul into
                    # free psum) instead of the ~1us gpsimd broadcast.
                    last = (g == NG - 1 and pair == 1)
                    for j in range(2):
                        qoff = j * 64
                        den = small.tile([1, 512], f32, tag="den")
                        if last:
                            # tail only: den staging on the now-idle ACT
                            # shortens the end-of-kernel serial chain (~1us)
                            nc.scalar.copy(den[:], oT[j][HD:HD + 1, :])
                        else:
                            nc.vector.tensor_copy(den[:], oT[j][HD:HD + 1, :])
                        rec = small.tile([1, 512], f32, tag="rec")
                        nc.vector.reciprocal_approx_fast(rec[:], den[:])
                        bcs = small.tile([64, 512], f32, tag="bcs")
                        nc.gpsimd.partition_broadcast(bcs[:], rec[:])
                        nc.vector.tensor_mul(xt[qoff:qoff + 64, qlo:qlo + 512],
                                             oT[j][:HD, :], bcs[:])

            rope_chunk_a(0)
            for g in range(NG):
                attn_group(g)
            # tail fillers: bridge the pair-B normalize window so the PE
            # clock stays at 8/8 for the final projection matmuls
            for _ in range(2):
                warm2 = psS.tile([128, 1024], f32, name="warm2", tag="S")
                for j in range(2):
                    nc.tensor.matmul(warm2[:, j * 512:(j + 1) * 512],
                                     dum[:, 0:128], dum[:],
                                     start=True, stop=True)
            # tail: group-3's w1@x1 half, spread across all three DMA rings
            for j in range(D // 128):
                st_eng = (nc.vector.tensor_copy if j % 2 == 0
                          else nc.scalar.copy)
                dma_eng = [nc.sync.dma_start, nc.scalar.dma_start,
                           nc.gpsimd.dma_start][j % 3]
                proj3_half(1, j, j % 4, st_eng, dma_eng)

    nc.finalize()
    return nc


def _get_nc():
    if "nc" not in _cache:
        _cache["nc"] = _build()
    return _cache["nc"]


def _in_maps(q, k, v, w_out):
    import ml_dtypes
    bf = ml_dtypes.bfloat16
    f8 = ml_dtypes.float8_e4m3
    cosT, sinT, rotP, mask01 = _tables()
    ones = np.ones((T, 1), np.float32)
    maps = []
    for c in range(NCORES):
        b, kv = divmod(c, NKV)
        va = np.ascontiguousarray(
            np.concatenate([v[b, :, kv * KVD:(kv + 1) * KVD], ones], 1))
        # device SBUF layout [p, kb, m]: row p holds k-position kb*128+p
        va_p = va.reshape(NKB, 128, HD + 1).transpose(1, 0, 2)
        va8_p = np.pad(va_p, ((0, 0), (0, 0), (0, 80 - (HD + 1))))
        qTc = np.ascontiguousarray(q[b, :, kv * CD:(kv + 1) * CD].T)
        kTc = np.ascontiguousarray(k[b, :, kv * KVD:(kv + 1) * KVD].T)
        head = np.zeros((128, 2816), np.float32)
        head[:, 0:512] = qTc[0:128, 0:512]
        head[:, 512:1024] = qTc[128:256, 0:512]
        head[:, 1024:1152] = rotP
        head[:, 1152:1280] = mask01
        head[0:64, 1280:1792] = kTc[:, 0:512]
        head[:, 1792:2304] = np.vstack([cosT[:, 0:512]] * 2)
        head[:, 2304:2816] = np.vstack([sinT[:, 0:512]] * 2)
        head2 = np.zeros((128, 2560), np.float32)
        head2[:, 0:512] = qTc[0:128, 512:1024]
        head2[:, 512:1024] = qTc[128:256, 512:1024]
        head2[0:64, 1024:1536] = kTc[:, 512:1024]
        head2[:, 1536:2048] = np.vstack([cosT[:, 512:1024]] * 2)
        head2[:, 2048:2560] = np.vstack([sinT[:, 512:1024]] * 2)
        maps.append({
            "qT": qTc.astype(bf),
            "kT": kTc.astype(bf),
            "head": head.astype(bf),
            "head2": head2.astype(bf),
            "vaug": np.ascontiguousarray(va_p.reshape(128, -1)).astype(bf),
            "vaug8": np.ascontiguousarray(
                np.clip(va8_p, -240, 240).reshape(128, -1)).astype(f8),
            "w": np.ascontiguousarray(w_out[kv * CD:(kv + 1) * CD, :]).astype(bf),
            "cosT": cosT.astype(bf), "sinT": sinT.astype(bf),
        })
    return maps


def _run(q, k, v, w_out, trace=False):
    from concourse.bass_utils import run_bass_kernel_spmd

    nc = _get_nc()
    res = run_bass_kernel_spmd(nc, _in_maps(q, k, v, w_out),
                               core_ids=list(range(NCORES)), trace=trace)
    out = np.zeros((B, T, D), np.float32)
    for c in range(NCORES):
        ot = res.results[c]["outT"].T.astype(np.float32)
        ot[3 * 512:] = 0.0   # q-cols 1536:2048 come from out3 instead
        out[c // NKV] += ot
        o3 = res.results[c]["out3"].astype(np.float32)
        out[c // NKV][3 * 512:] += (o3[0] + o3[1]).T
    return out, res


def kernel(q, k, v, w_out):
    out, _ = _run(np.asarray(q), np.asarray(k), np.asarray(v),
                  np.asarray(w_out))
    return out



# revision 53
# speedup vs baseline: 1.0022x; 1.0022x over previous
"""GQA attention block (RoPE + causal softmax + out-projection) on 8 TRN2 cores.

Problem: q (2, 2048, 1024) 16 heads, k/v (2, 2048, 256) 4 kv heads (GQA rep 4),
causal attention, out @ w_out (1024, 1024).

Sharding: core c = (batch b = c//4, kv group = c%4). Each core computes its 4
q-heads x full T attention against its kv head, then the partial projection
X_heads @ w_out[head_rows, :]; the host sums the 4 partials per batch.

Layout: everything computed transposed (channels on partitions, sequence on
the free axis). Per 512-wide q group, the 4 q heads are processed as two
sequential PAIRS (even head at partitions 0-63, odd at 64-127):
  - S^T for a pair lives in one [128, 1024] psum tile (2 banks, 2-buf).
    exp(S/8 - 2.5) is one ACT instruction per round (ACT is the pacing
    engine, ~78us of exp); the -2.5 bias keeps fp8 P under 240 and cancels
    in the softmax division.
  - SUB-DIAGONAL rounds: exp emits fp8e4 P^T into [128, 2, 1024] pair
    tiles; O^T accumulates via fp8 DoubleRow matmuls (2 k-blocks per pass,
    0.5 cyc/row; vaug8 padded to 80 cols for the LDWEIGHTS stride%16 rule).
    DIAGONAL rounds stay bf16 (regular matmuls) so the dominant
    near-diagonal weights keep precision: rel err 2.9e-3 vs 2.2e-2 all-fp8.
  - O matmuls are EMITTED 2 rounds late (pend/flush): the in-order PE FIFO
    otherwise head-of-line blocks S(kb+1) behind exp(kb).
  - causal masking: diagonal 128x128 strip masked MULTIPLICATIVELY on bf16
    P^T after exp (psum-accumulate masks silently break on HW).
  - O^T accumulates [V | 1] stationary; psum row 64 is the denominator.
    Normalize per head: den copy (DVE; ACT in the tail), 
    reciprocal_approx_fast (SBUF-only input), GpSimd partition_broadcast,
    one multiply.
  - projection (chunk n: w[:,n*128:+128]^T @ X^T) is DEFERRED into the next
    group's pair-A rounds (psum borrows the idle pair-B oT banks); group
    3's projection is split per channel-half into d_out3 partials so half
    streams out during pair B. Output DMAs alternate the SP/GpSimd rings.
  - RoPE per 512-col chunk, interleaved so each group's q/k are ready well
    before its first S matmul; independent muls emitted before
    psum-dependent ops (the DVE chain is the startup critical path).
  - input DMAs: SDMA engines drain descriptors in issue order and pay a
    fixed cost per descriptor, so chunk-0 and chunk-1 rope inputs are
    host-packed into two [128, ~2.7K] "head" slabs (one DMA, 128
    descriptors each) issued before the bulk; remaining loads are
    whole-tensor, need-ordered across the SP/ACT/GpSimd rings.
  - HAM: warmup burst + keep-warm fillers in the sparse early groups hold
    the PE at 2.4 GHz.
rotate_half is a signed-permutation matmul; sin/cos tables are host-built
[64, T] (rows repeat with period 32, k-heads use rows 0-63).
"""

import sys

if "/opt/trn_rl_repo" not in sys.path:
    sys.path.insert(0, "/opt/trn_rl_repo")

import numpy as np

B, T, D, NH, NKV, HD = 2, 2048, 1024, 16, 4, 64
HC = NH // NKV          # q heads per core = 4
CD = HC * HD            # per-core channel dim = 256
KVD = HD                # per-core kv channel dim = 64
NCORES = 8
QB = 128                # q/k block
NG = T // 512           # q column groups of 512
NKB = T // QB           # k blocks = 16
MASK = -240.0           # pre-scale additive mask; exp(-240/8) = exp(-30) ~ 1e-13
EBIAS = -2.5            # exp bias shift: keeps fp8 P in [~0, 104] < 240 (fp8e4
                        # overflows to Inf above 240); cancels in softmax

_cache: dict = {}


def _tables():
    if "tables" in _cache:
        return _cache["tables"]
    p = np.arange(64)
    t = np.arange(T)
    ang = t[None, :] / (10000.0 ** ((p[:, None] % 32) / 32.0))
    cosT = np.cos(ang).astype(np.float32)   # [64, T]
    sinT = np.sin(ang).astype(np.float32)

    rotP = np.zeros((128, 128), np.float32)
    for base in (0, 64):
        for i in range(32):
            rotP[base + 32 + i, base + i] = -1.0   # out[i] = -x[i+32]
            rotP[base + i, base + 32 + i] = 1.0    # out[i+32] = x[i]

    kk = np.arange(QB)
    # multiplicative causal mask for the diagonal 128x128 strip: P *= mask01
    mask01 = np.where(kk[:, None] <= kk[None, :], 1.0, 0.0).astype(np.float32)
    _cache["tables"] = (cosT, sinT, rotP, mask01)
    return _cache["tables"]


def _build():
    import concourse.tile as tile
    from concourse import bacc, mybir

    f32 = mybir.dt.float32
    bf16 = mybir.dt.bfloat16
    f8 = mybir.dt.float8e4
    Exp = mybir.ActivationFunctionType.Exp
    DR = mybir.MatmulPerfMode.DoubleRow

    nc = bacc.Bacc("TRN2", target_bir_lowering=False, debug=False,
                   num_devices=NCORES)

    d_qT = nc.dram_tensor("qT", [CD, T], bf16, kind="ExternalInput")
    d_kT = nc.dram_tensor("kT", [KVD, T], bf16, kind="ExternalInput")
    d_vaug = nc.dram_tensor("vaug", [128, NKB * (HD + 1)], bf16,
                        kind="ExternalInput")
    # fp8 V padded to 80 cols: DoubleRow LDWEIGHTS requires the k-subtile
    # stride to be a multiple of 16 (s3_lw_dual_fp8_restrictions). Both V
    # slabs are host-pre-arranged to the SBUF layout so the input DMA is one
    # contiguous per-partition strip (a (n p) m gather costs 1-6us of HWDGE
    # descriptor-issue time on the ring).
    d_vaug8 = nc.dram_tensor("vaug8", [128, NKB * 80], f8,
                             kind="ExternalInput")
    d_w = nc.dram_tensor("w", [CD, D], bf16, kind="ExternalInput")
    d_cosT = nc.dram_tensor("cosT", [64, T], bf16, kind="ExternalInput")
    d_sinT = nc.dram_tensor("sinT", [64, T], bf16, kind="ExternalInput")
    # packed "head" slab: everything rope chunk 0 needs in ONE DMA (SDMA
    # pays ~0.4us per 8-descriptor engine queue — six separate head loads
    # cost ~500 descriptors and land at 12-15us; one slab = 128, lands ~10)
    # layout per partition: [q0c0 512 | q1c0 512 | rotP 128 | mask 128 |
    #                        kc0 512 (lo only) | cos_c0 512 | sin_c0 512]
    d_head = nc.dram_tensor("head", [128, 2816], bf16, kind="ExternalInput")
    # chunk-1 slab: [q0c1 512 | q1c1 512 | kc1 512 (lo) | cos_c1 | sin_c1]
    d_head2 = nc.dram_tensor("head2", [128, 2560], bf16, kind="ExternalInput")
    d_outT = nc.dram_tensor("outT", [D, T], bf16, kind="ExternalOutput")
    # group-3 projection partials (w0@x0 and w1@x1 halves, host-summed) so
    # half the tail projection can stream out during pair B
    d_out3 = nc.dram_tensor("out3", [2, D, 512], bf16, kind="ExternalOutput")

    with tile.TileContext(nc) as tc:
        with (
            tc.tile_pool(name="consts", bufs=1) as consts,
            tc.tile_pool(name="data", bufs=1) as data,
            tc.tile_pool(name="pt", bufs=8) as ptp,
            tc.tile_pool(name="small", bufs=6) as small,
            tc.tile_pool(name="psS", bufs=2, space="PSUM") as psS,
            tc.tile_pool(name="psO", bufs=1, space="PSUM") as psO,
        ):
            cosT = consts.tile([128, T], bf16)
            sinT = consts.tile([128, T], bf16)
            head = consts.tile([128, 2816], bf16)
            HQ0, HQ1, HRP, HMK, HKC, HCS, HSN = (0, 512, 1024, 1152, 1280,
                                                 1792, 2304)
            head2 = consts.tile([128, 2560], bf16)
            JQ0, JQ1, JKC, JCS, JSN = 0, 512, 1024, 1536, 2048
            ebias = consts.tile([128, 1], f32)
            nc.vector.memset(ebias[:], EBIAS)
            nc.sync.dma_start(head[:], d_head[:])

            qT = [data.tile([128, T], bf16, name=f"qT{i}", tag=f"qT{i}")
                  for i in range(2)]
            kT = data.tile([KVD, T], bf16, tag="kT")
            kThi = data.tile([128, T], bf16, tag="kThi")
            vaug = data.tile([128, NKB, HD + 1], bf16, tag="vaug")
            vaug8 = data.tile([128, NKB, 80], f8, tag="vaug8")
            w = [data.tile([128, D], bf16, name=f"w{i}", tag=f"w{i}")
                 for i in range(2)]
            xT = [data.tile([128, T], bf16, name=f"xT{i}", tag=f"xT{i}")
                  for i in range(2)]

            # tiny exp reading the locally-memset tile (no DMA dep) so
            # walrus's ACT_TABLE_LOAD runs immediately, before the ACT-ring
            # input DMAs occupy the queue
            dum = data.tile([128, 512], bf16, tag="dum")
            nc.vector.memset(dum[:], 1.0)
            # ones column for the tail's PE-side reciprocal broadcast
            onesc = data.tile([1, 64], f32, tag="onesc")
            nc.vector.memset(onesc[:], 1.0)
            dummy = small.tile([1, 8], bf16, tag="dummy")
            nc.scalar.activation(dummy[:], dum[0:1, 0:8], Exp, scale=0.125)

            # PE warmup: DMA-independent dense burst so HAM reaches full
            # clock early; 16 x N=512 cold matmuls (~7us) guarantee covering
            # a full free-running HAM activity window
            for _ in range(5):
                warm = psS.tile([128, 1024], f32, name="warm", tag="S")
                for j in range(2):
                    nc.tensor.matmul(warm[:, j * 512:(j + 1) * 512],
                                     dum[:, 0:128], dum[:],
                                     start=True, stop=True)

            # whole-tensor input DMAs. HWDGE issue is serial per ring
            # (~650ns each), so the rope-chunk-0 deps are split across BOTH
            # HWDGE rings (SP + ACT — ACT is idle until the first exp at
            # ~13us anyway); bulky contiguous loads ride the GpSimd SWDGE
            # ring. Ring order = first-use order.
            # Input DMA order = SDMA drain order = need order. After the
            # head slab: tails (cols 512:T) of q/k/cos/sin (rope chunk 1 at
            # ~12.5us needs their first third), then late consumers.
            tl = slice(1024, T)
            nc.sync.dma_start(head2[:], d_head2[:])
            nc.sync.dma_start(qT[0][:, tl], d_qT[0:128, tl])
            nc.sync.dma_start(kT[:, tl], d_kT[:, tl])
            nc.sync.dma_start(cosT[0:64, tl], d_cosT[:, tl])
            nc.sync.dma_start(sinT[0:64, tl], d_sinT[:, tl])
            nc.scalar.dma_start(cosT[64:128, tl], d_cosT[:, tl])
            nc.scalar.dma_start(sinT[64:128, tl], d_sinT[:, tl])
            nc.sync.dma_start(qT[1][:, tl], d_qT[128:256, tl])
            nc.gpsimd.dma_start(
                vaug[:].rearrange("p n m -> p (n m)"), d_vaug[:])
            nc.gpsimd.dma_start(
                vaug8[:].rearrange("p n m -> p (n m)"), d_vaug8[:])
            nc.gpsimd.dma_start(w[0][:], d_w[0:128, :])
            nc.gpsimd.dma_start(w[1][:], d_w[128:256, :])

            def rope_chunk_a(f, tq="oT0", tk="oT1"):
                # psum scratch borrows idle oT banks (oT2/3 during pair A
                # after the proj chunks; oT0/1 during prologue/pair B); the S
                # pool is never touched so the exp pipeline keeps its 2 slots.
                # Part a: qT[0] + kT (all the next pair A needs).
                sl = slice(f * 512, (f + 1) * 512)
                if f == 0:
                    qs = head[:, HQ0:HQ0 + 512]
                    ks = head[0:64, HKC:HKC + 512]
                    co, si = head[:, HCS:HCS + 512], head[:, HSN:HSN + 512]
                    co64 = head[0:64, HCS:HCS + 512]
                    si64 = head[0:64, HSN:HSN + 512]
                elif f == 1:
                    qs = head2[:, JQ0:JQ0 + 512]
                    ks = head2[0:64, JKC:JKC + 512]
                    co, si = head2[:, JCS:JCS + 512], head2[:, JSN:JSN + 512]
                    co64 = head2[0:64, JCS:JCS + 512]
                    si64 = head2[0:64, JSN:JSN + 512]
                else:
                    qs, ks = qT[0][:, sl], kT[:, sl]
                    co, si = cosT[:, sl], sinT[:, sl]
                    co64, si64 = cosT[:KVD, sl], sinT[:KVD, sl]
                rk = psO.tile([128, 512], f32, name="rk", tag=tk)
                nc.tensor.matmul(rk[:KVD, :], head[0:64, HRP:HRP + 64],
                                 ks, start=True, stop=True)
                rq0 = psO.tile([128, 512], f32, name="rq0", tag=tq)
                nc.tensor.matmul(rq0[:], head[:, HRP:HRP + 128], qs,
                                 start=True, stop=True)
                # independent muls first, psum-dependent ops after — the DVE
                # chain is the startup critical path
                nc.vector.tensor_mul(kT[:, sl], ks, co64)
                nc.vector.tensor_mul(qT[0][:, sl], qs, co)
                nc.vector.tensor_mul(rk[:KVD, :], rk[:KVD, :], si64)
                nc.vector.tensor_add(kT[:, sl], kT[:, sl], rk[:KVD, :])
                nc.vector.tensor_copy(kThi[64:128, sl], kT[:, sl])
                nc.vector.tensor_mul(rq0[:], rq0[:], si)
                nc.vector.tensor_add(qT[0][:, sl], qT[0][:, sl], rq0[:])

            def rope_chunk_b(f, t="oT0"):
                # Part b: qT[1]; emitted a few rounds after part a so the
                # rq1 matmul's WAR wait on the rq0 slot is already satisfied
                # (an unsatisfied wait here would stall the whole PE FIFO).
                sl = slice(f * 512, (f + 1) * 512)
                if f == 0:
                    qs = head[:, HQ1:HQ1 + 512]
                    co, si = head[:, HCS:HCS + 512], head[:, HSN:HSN + 512]
                elif f == 1:
                    qs = head2[:, JQ1:JQ1 + 512]
                    co, si = head2[:, JCS:JCS + 512], head2[:, JSN:JSN + 512]
                else:
                    qs = qT[1][:, sl]
                    co, si = cosT[:, sl], sinT[:, sl]
                rq1 = psO.tile([128, 512], f32, name="rq1", tag=t)
                nc.tensor.matmul(rq1[:], head[:, HRP:HRP + 128], qs,
                                 start=True, stop=True)
                nc.vector.tensor_mul(rq1[:], rq1[:], si)
                nc.vector.tensor_mul(qT[1][:, sl], qs, co)
                nc.vector.tensor_add(qT[1][:, sl], qT[1][:, sl], rq1[:])

            def proj_chunk(gq, j, slot):
                # out^T[n-chunk j, qlo:qlo+512] = sum_cc w[cc]^T @ xT[cc];
                # output DMAs alternate the SP and GpSimd rings
                qlo = gq * 512
                pr = psO.tile([128, 512], f32, name="pr", tag=f"oT{slot}")
                for cc in range(2):
                    nc.tensor.matmul(pr[:], w[cc][:, j * 128:(j + 1) * 128],
                                     xT[cc][:, qlo:qlo + 512],
                                     start=(cc == 0), stop=(cc == 1))
                st = ptp.tile([128, 512], bf16, name="st", tag="st")
                nc.vector.tensor_copy(st[:], pr[:])
                eng = nc.sync if j % 2 == 0 else nc.gpsimd
                eng.dma_start(d_outT[j * 128:(j + 1) * 128, qlo:qlo + 512],
                              st[:])

            def proj3_half(cc, j, slot, st_eng, dma_eng):
                # group-3 projection, single-cc half: streams to d_out3[cc]
                pr = psO.tile([128, 512], f32, name="pr3", tag=f"oT{slot}")
                nc.tensor.matmul(pr[:], w[cc][:, j * 128:(j + 1) * 128],
                                 xT[cc][:, 3 * 512:4 * 512],
                                 start=True, stop=True)
                st = ptp.tile([128, 512], bf16, name="st3", tag="st")
                st_eng(st[:], pr[:])
                dma_eng(d_out3[cc, j * 128:(j + 1) * 128, :], st[:])

            carry = {}

            def attn_group(g):
                qlo = g * 512
                nkb = (qlo + 512) // QB
                kb0 = qlo // QB
                for pair in range(2):
                    qt = qT[pair]
                    xt = xT[pair]
                    if g == 1 and pair == 0 and "oT" in carry:
                        # sub-diagonal rounds kb0-3 already ran, interleaved
                        # into g0's exp-sparse pair B; continue their oT
                        # accumulation from kb4 (diag rounds)
                        oT = carry.pop("oT")
                        kb_start = 4
                    else:
                        oT = [psO.tile([HD + 1, 512], f32,
                                       name=f"oT{2*pair+j}",
                                       tag=f"oT{2*pair+j}") for j in range(2)]
                        kb_start = 0
                    PTd = None
                    # O matmuls are EMITTED 2 rounds late: in-order PE FIFO
                    # means an O placed right after its exp head-of-line
                    # blocks S(kb+1) behind exp(kb) — round time becomes
                    # S+exp+O serial instead of exp-paced. Two rounds of
                    # slack (the S-pool WAR already keeps PE <=2 rounds
                    # ahead) guarantees the O's inputs are long done.
                    pend = []

                    def flush_pend(upto_kb):
                        while pend and pend[0][0] <= upto_kb:
                            pend.pop(0)[1]()
                    for kb in range(kb_start, nkb):
                        diag = kb >= kb0
                        cs = QB * (kb - kb0) if diag else 0
                        S2 = psS.tile([128, 1024], f32, name="S2", tag="S")
                        S2v = S2.rearrange("p (j n) -> p j n", j=2)
                        if g <= 1:
                            # keep-warm filler: early groups have too little
                            # PE work per round and HAM re-throttles the PE
                            # clock to 1.2 GHz without it
                            nc.tensor.matmul(S2[:, 0:128], dum[:, 0:128],
                                             dum[:, 0:128],
                                             start=True, stop=True)
                        for j in range(2):
                            qoff = j * 64
                            kTh = kT if j == 0 else kThi[64:128, :]
                            base = j * 512
                            nc.tensor.matmul(
                                S2[:, base + cs:base + 512],
                                kTh[:, kb * QB:(kb + 1) * QB],
                                qt[qoff:qoff + 64, qlo + cs:qlo + 512],
                                start=True, stop=True)
                        flush_pend(kb - 3)
                        if not diag:
                            # sub-diagonal rounds: exp straight to fp8 and
                            # accumulate O via DoubleRow (2 k-blocks per pass,
                            # 0.5 cyc/row) — diagonal strip stays bf16 so the
                            # dominant near-diagonal weights keep precision
                            if kb % 2 == 0:
                                PTd = ptp.tile([128, 2, 1024], f8,
                                               name="PTd", tag="PTd")
                            nc.scalar.activation(PTd[:, kb % 2, :], S2[:],
                                                 Exp, scale=0.125, bias=ebias[:])
                            if kb % 2 == 1:
                                def odr(kb=kb, PTd=PTd):
                                    for j in range(2):
                                        nc.tensor.matmul(
                                            oT[j][:, :],
                                            vaug8[:, kb - 1:kb + 1, 0:HD + 1],
                                            PTd[:, :, j * 512:(j + 1) * 512],
                                            start=(kb == 1), stop=False,
                                            perf_mode=DR,
                                            skip_group_check=True)
                                pend.append((kb, odr))
                        else:
                            PT = ptp.tile([128, 1024], bf16, name="PT",
                                          tag="PT")
                            if cs:
                                PTv = PT.rearrange("p (j n) -> p j n", j=2)
                                nc.scalar.activation(PTv[:, :, cs:],
                                                     S2v[:, :, cs:],
                                                     Exp, scale=0.125,
                                                     bias=ebias[:])
                            else:
                                nc.scalar.activation(PT[:], S2[:], Exp,
                                                     scale=0.125, bias=ebias[:])
                            # multiplicative causal mask on the diagonal
                            # 128-wide strip of each head's P^T (no psum
                            # has_written reliance — robust on HW)
                            for j in range(2):
                                nc.vector.tensor_mul(
                                    PT[:, j * 512 + cs:j * 512 + cs + QB],
                                    PT[:, j * 512 + cs:j * 512 + cs + QB],
                                    head[:, HMK:HMK + QB])

                            def odiag(kb=kb, cs=cs, PT=PT):
                                for j in range(2):
                                    nc.tensor.matmul(
                                        oT[j][:, cs:], vaug[:, kb, :],
                                        PT[:, j * 512 + cs:(j + 1) * 512],
                                        start=(kb == 0), stop=(kb == nkb - 1),
                                        skip_group_check=True)
                            pend.append((kb, odiag))
                        # deferred work hooks. For g1/g2 the projection
                        # burst is split across BOTH pairs (4 chunks each) —
                        # 8 chunks in pair A's first rounds starve ACT for
                        # ~4.6us; pair B's kb2/kb4 use the freed oT0/1 slots
                        if pair == 0 and g == 2 and 1 <= kb <= 4:
                            proj_chunk(g - 1, kb - 1, 2 + (kb % 2))
                        if pair == 0 and g == 1 and 4 <= kb <= 7:
                            proj_chunk(g - 1, kb - 4, 2 + (kb % 2))
                        if pair == 0 and g == 3 and 1 <= kb <= 8:
                            proj_chunk(g - 1, kb - 1, 2 + (kb % 2))
                        if pair == 1 and 1 <= g <= 2 and kb in (2, 4):
                            for i in range(2):
                                proj_chunk(g - 1, 4 + (kb // 2 - 1) * 2 + i, i)
                        if pair == 0 and 1 <= g <= 2 and kb == 6:
                            # rope the next chunk early (pair A, borrowing
                            # the proj banks) so the DVE backlog of pair B
                            # can't delay the next group's q/k readiness
                            rope_chunk_a(g + 1, tq="oT2", tk="oT3")
                        if pair == 0 and g == 0 and kb == 0:
                            # q1 chunk 0 rides the head slab (lands ~10us);
                            # roping it here keeps the prologue FIFOs clear
                            rope_chunk_b(0, t="oT2")
                        if pair == 0 and g == 0 and kb == 1:
                            rope_chunk_a(1, tq="oT2", tk="oT3")
                        if pair == 0 and g == 0 and kb == 3:
                            rope_chunk_b(1, t="oT2")
                        if pair == 1 and kb == min(3, nkb - 2) and 1 <= g <= 2:
                            rope_chunk_b(g + 1)
                        if pair == 1 and g == 0 and kb in (2, 3):
                            # interleave g1 pair-A's sub-diagonal rounds here:
                            # g0 pair B gives ACT only 4 narrow diag exps, and
                            # q chunk 1 / k chunk 0 / vaug8 are all ready
                            if kb == 2:
                                carry["oT"] = [
                                    psO.tile([HD + 1, 512], f32,
                                             name=f"oTn{j}", tag=f"oT{j}")
                                    for j in range(2)]
                            kbb = (kb - 2) * 2
                            PTdn = ptp.tile([128, 2, 1024], f8,
                                            name="PTdn", tag="PTd")
                            for kk in (kbb, kbb + 1):
                                S2n = psS.tile([128, 1024], f32,
                                               name="S2n", tag="S")
                                for j in range(2):
                                    kTh2 = kT if j == 0 else kThi[64:128, :]
                                    nc.tensor.matmul(
                                        S2n[:, j * 512:(j + 1) * 512],
                                        kTh2[:, kk * QB:(kk + 1) * QB],
                                        qT[0][j * 64:j * 64 + 64, 512:1024],
                                        start=True, stop=True)
                                nc.scalar.activation(PTdn[:, kk % 2, :],
                                                     S2n[:], Exp,
                                                     scale=0.125,
                                                     bias=ebias[:])
                            for j in range(2):
                                nc.tensor.matmul(
                                    carry["oT"][j][:, :],
                                    vaug8[:, kbb:kbb + 2, 0:HD + 1],
                                    PTdn[:, :, j * 512:(j + 1) * 512],
                                    start=(kbb == 0), stop=False,
                                    perf_mode=DR,
                                    skip_group_check=True)
                        if pair == 1 and g == 3 and 4 <= kb <= 11:
                            # stream group-3's w0@x0 projection half during
                            # pair B (oT0/oT1 banks are free: no rope here)
                            j3 = kb - 4
                            proj3_half(0, j3, j3 % 2,
                                       nc.vector.tensor_copy,
                                       nc.gpsimd.dma_start)
                    flush_pend(nkb)
                    # normalize the pair; reciprocal_approx_fast needs SBUF
                    # input (PSUM source diverges on HW), so stage the
                    # denominator row first. In the tail (last pair) the
                    # broadcast runs on the idle PE (ones-column matmul into
                    # free psum) instead of the ~1us gpsimd broadcast.
                    last = (g == NG - 1 and pair == 1)
                    for j in range(2):
                        qoff = j * 64
                        den = small.tile([1, 512], f32, tag="den")
                        if last:
                            # tail only: den staging on the now-idle ACT
                            # shortens the end-of-kernel serial chain (~1us)
                            nc.scalar.copy(den[:], oT[j][HD:HD + 1, :])
                        else:
                            nc.vector.tensor_copy(den[:], oT[j][HD:HD + 1, :])
                        rec = small.tile([1, 512], f32, tag="rec")
                        nc.vector.reciprocal_approx_fast(rec[:], den[:])
                        bcs = small.tile([64, 512], f32, tag="bcs")
                        nc.gpsimd.partition_broadcast(bcs[:], rec[:])
                        nc.vector.tensor_mul(xt[qoff:qoff + 64, qlo:qlo + 512],
                                             oT[j][:HD, :], bcs[:])

            rope_chunk_a(0)
            for g in range(NG):
                attn_group(g)
            # tail fillers: bridge the pair-B normalize window so the PE
            # clock stays at 8/8 for the final projection matmuls
            for _ in range(2):
                warm2 = psS.tile([128, 1024], f32, name="warm2", tag="S")
                for j in range(2):
                    nc.tensor.matmul(warm2[:, j * 512:(j + 1) * 512],
                                     dum[:, 0:128], dum[:],
                                     start=True, stop=True)
            # tail: group-3's w1@x1 half, spread across all three DMA rings
            for j in range(D // 128):
                st_eng = (nc.vector.tensor_copy if j % 2 == 0
                          else nc.scalar.copy)
                dma_eng = [nc.sync.dma_start, nc.scalar.dma_start,
                           nc.gpsimd.dma_start][j % 3]
                proj3_half(1, j, j % 4, st_eng, dma_eng)

    nc.finalize()
    return nc


def _get_nc():
    if "nc" not in _cache:
        _cache["nc"] = _build()
    return _cache["nc"]


def _in_maps(q, k, v, w_out):
    import ml_dtypes
    bf = ml_dtypes.bfloat16
    f8 = ml_dtypes.float8_e4m3
    cosT, sinT, rotP, mask01 = _tables()
    ones = np.ones((T, 1), np.float32)
    maps = []
    for c in range(NCORES):
        b, kv = divmod(c, NKV)
        va = np.ascontiguousarray(
            np.concatenate([v[b, :, kv * KVD:(kv + 1) * KVD], ones], 1))
        # device SBUF layout [p, kb, m]: row p holds k-position kb*128+p
        va_p = va.reshape(NKB, 128, HD + 1).transpose(1, 0, 2)
        va8_p = np.pad(va_p, ((0, 0), (0, 0), (0, 80 - (HD + 1))))
        qTc = np.ascontiguousarray(q[b, :, kv * CD:(kv + 1) * CD].T)
        kTc = np.ascontiguousarray(k[b, :, kv * KVD:(kv + 1) * KVD].T)
        head = np.zeros((128, 2816), np.float32)
        head[:, 0:512] = qTc[0:128, 0:512]
        head[:, 512:1024] = qTc[128:256, 0:512]
        head[:, 1024:1152] = rotP
        head[:, 1152:1280] = mask01
        head[0:64, 1280:1792] = kTc[:, 0:512]
        head[:, 1792:2304] = np.vstack([cosT[:, 0:512]] * 2)
        head[:, 2304:2816] = np.vstack([sinT[:, 0:512]] * 2)
        head2 = np.zeros((128, 2560), np.float32)
        head2[:, 0:512] = qTc[0:128, 512:1024]
        head2[:, 512:1024] = qTc[128:256, 512:1024]
        head2[0:64, 1024:1536] = kTc[:, 512:1024]
        head2[:, 1536:2048] = np.vstack([cosT[:, 512:1024]] * 2)
        head2[:, 2048:2560] = np.vstack([sinT[:, 512:1024]] * 2)
        maps.append({
            "qT": qTc.astype(bf),
            "kT": kTc.astype(bf),
            "head": head.astype(bf),
            "head2": head2.astype(bf),
            "vaug": np.ascontiguousarray(va_p.reshape(128, -1)).astype(bf),
            "vaug8": np.ascontiguousarray(
                np.clip(va8_p, -240, 240).reshape(128, -1)).astype(f8),
            "w": np.ascontiguousarray(w_out[kv * CD:(kv + 1) * CD, :]).astype(bf),
            "cosT": cosT.astype(bf), "sinT": sinT.astype(bf),
        })
    return maps


def _run(q, k, v, w_out, trace=False):
    from concourse.bass_utils import run_bass_kernel_spmd

    nc = _get_nc()
    res = run_bass_kernel_spmd(nc, _in_maps(q, k, v, w_out),
                               core_ids=list(range(NCORES)), trace=trace)
    out = np.zeros((B, T, D), np.float32)
    for c in range(NCORES):
        ot = res.results[c]["outT"].T.astype(np.float32)
        ot[3 * 512:] = 0.0   # q-cols 1536:2048 come from out3 instead
        out[c // NKV] += ot
        o3 = res.results[c]["out3"].astype(np.float32)
        out[c // NKV][3 * 512:] += (o3[0] + o3[1]).T
    return out, res


def kernel(q, k, v, w_out):
    out, _ = _run(np.asarray(q), np.asarray(k), np.asarray(v),
                  np.asarray(w_out))
    return out



# revision 54
# speedup vs baseline: 1.0179x; 1.0156x over previous
"""GQA attention block (RoPE + causal softmax + out-projection) on 8 TRN2 cores.

Problem: q (2, 2048, 1024) 16 heads, k/v (2, 2048, 256) 4 kv heads (GQA rep 4),
causal attention, out @ w_out (1024, 1024).

Sharding: core c = (batch b = c//4, kv group = c%4). Each core computes its 4
q-heads x full T attention against its kv head, then the partial projection
X_heads @ w_out[head_rows, :]; the host sums the 4 partials per batch.

Layout: everything computed transposed (channels on partitions, sequence on
the free axis). Per 512-wide q group, the 4 q heads are processed as two
sequential PAIRS (even head at partitions 0-63, odd at 64-127):
  - S^T for a pair lives in one [128, 1024] psum tile (2 banks, 2-buf).
    exp(S/8 - 2.5) is one ACT instruction per round (ACT is the pacing
    engine, ~78us of exp); the -2.5 bias keeps fp8 P under 240 and cancels
    in the softmax division.
  - SUB-DIAGONAL rounds: exp emits fp8e4 P^T into [128, 2, 1024] pair
    tiles; O^T accumulates via fp8 DoubleRow matmuls (2 k-blocks per pass,
    0.5 cyc/row; vaug8 padded to 80 cols for the LDWEIGHTS stride%16 rule).
    DIAGONAL rounds stay bf16 (regular matmuls) so the dominant
    near-diagonal weights keep precision: rel err 2.9e-3 vs 2.2e-2 all-fp8.
  - O matmuls are EMITTED 2 rounds late (pend/flush): the in-order PE FIFO
    otherwise head-of-line blocks S(kb+1) behind exp(kb).
  - causal masking: diagonal 128x128 strip masked MULTIPLICATIVELY on bf16
    P^T after exp (psum-accumulate masks silently break on HW).
  - O^T accumulates [V | 1] stationary; psum row 64 is the denominator.
    Normalize per head: den copy (DVE; ACT in the tail), 
    reciprocal_approx_fast (SBUF-only input), GpSimd partition_broadcast,
    one multiply.
  - projection (chunk n: w[:,n*128:+128]^T @ X^T) is DEFERRED into the next
    group's pair-A rounds (psum borrows the idle pair-B oT banks); group
    3's projection is split per channel-half into d_out3 partials so half
    streams out during pair B. Output DMAs alternate the SP/GpSimd rings.
  - RoPE per 512-col chunk, interleaved so each group's q/k are ready well
    before its first S matmul; independent muls emitted before
    psum-dependent ops (the DVE chain is the startup critical path).
  - input DMAs: SDMA engines drain descriptors in issue order and pay a
    fixed cost per descriptor, so chunk-0 and chunk-1 rope inputs are
    host-packed into two [128, ~2.7K] "head" slabs (one DMA, 128
    descriptors each) issued before the bulk; remaining loads are
    whole-tensor, need-ordered across the SP/ACT/GpSimd rings.
  - HAM: warmup burst + keep-warm fillers in the sparse early groups hold
    the PE at 2.4 GHz.
rotate_half is a signed-permutation matmul; sin/cos tables are host-built
[64, T] (rows repeat with period 32, k-heads use rows 0-63).
"""

import sys

if "/opt/trn_rl_repo" not in sys.path:
    sys.path.insert(0, "/opt/trn_rl_repo")

import numpy as np

B, T, D, NH, NKV, HD = 2, 2048, 1024, 16, 4, 64
HC = NH // NKV          # q heads per core = 4
CD = HC * HD            # per-core channel dim = 256
KVD = HD                # per-core kv channel dim = 64
NCORES = 8
QB = 128                # q/k block
NG = T // 512           # q column groups of 512
NKB = T // QB           # k blocks = 16
MASK = -240.0           # pre-scale additive mask; exp(-240/8) = exp(-30) ~ 1e-13
EBIAS = -2.5            # exp bias shift: keeps fp8 P in [~0, 104] < 240 (fp8e4
                        # overflows to Inf above 240); cancels in softmax

_cache: dict = {}


def _tables():
    if "tables" in _cache:
        return _cache["tables"]
    p = np.arange(64)
    t = np.arange(T)
    ang = t[None, :] / (10000.0 ** ((p[:, None] % 32) / 32.0))
    cosT = np.cos(ang).astype(np.float32)   # [64, T]
    sinT = np.sin(ang).astype(np.float32)

    rotP = np.zeros((128, 128), np.float32)
    for base in (0, 64):
        for i in range(32):
            rotP[base + 32 + i, base + i] = -1.0   # out[i] = -x[i+32]
            rotP[base + i, base + 32 + i] = 1.0    # out[i+32] = x[i]

    kk = np.arange(QB)
    # multiplicative causal mask for the diagonal 128x128 strip: P *= mask01
    mask01 = np.where(kk[:, None] <= kk[None, :], 1.0, 0.0).astype(np.float32)
    _cache["tables"] = (cosT, sinT, rotP, mask01)
    return _cache["tables"]


def _build():
    import concourse.tile as tile
    from concourse import bacc, mybir

    f32 = mybir.dt.float32
    bf16 = mybir.dt.bfloat16
    f8 = mybir.dt.float8e4
    Exp = mybir.ActivationFunctionType.Exp
    DR = mybir.MatmulPerfMode.DoubleRow

    nc = bacc.Bacc("TRN2", target_bir_lowering=False, debug=False,
                   num_devices=NCORES)

    d_qT = nc.dram_tensor("qT", [CD, T], bf16, kind="ExternalInput")
    d_kT = nc.dram_tensor("kT", [KVD, T], bf16, kind="ExternalInput")
    d_vaug = nc.dram_tensor("vaug", [128, NKB * (HD + 1)], bf16,
                        kind="ExternalInput")
    # fp8 V padded to 80 cols: DoubleRow LDWEIGHTS requires the k-subtile
    # stride to be a multiple of 16 (s3_lw_dual_fp8_restrictions). Both V
    # slabs are host-pre-arranged to the SBUF layout so the input DMA is one
    # contiguous per-partition strip (a (n p) m gather costs 1-6us of HWDGE
    # descriptor-issue time on the ring).
    d_vaug8 = nc.dram_tensor("vaug8", [128, NKB * 80], f8,
                             kind="ExternalInput")
    d_w = nc.dram_tensor("w", [CD, D], bf16, kind="ExternalInput")
    d_cosT = nc.dram_tensor("cosT", [64, T], bf16, kind="ExternalInput")
    d_sinT = nc.dram_tensor("sinT", [64, T], bf16, kind="ExternalInput")
    # packed "head" slab: everything rope chunk 0 needs in ONE DMA (SDMA
    # pays ~0.4us per 8-descriptor engine queue — six separate head loads
    # cost ~500 descriptors and land at 12-15us; one slab = 128, lands ~10)
    # layout per partition: [q0c0 512 | q1c0 512 | rotP 128 | mask 128 |
    #                        kc0 512 (lo only) | cos_c0 512 | sin_c0 512]
    d_head = nc.dram_tensor("head", [128, 2816], bf16, kind="ExternalInput")
    # chunk-1 slab: [q0c1 512 | q1c1 512 | kc1 512 (lo) | cos_c1 | sin_c1]
    d_head2 = nc.dram_tensor("head2", [128, 2560], bf16, kind="ExternalInput")
    d_outT = nc.dram_tensor("outT", [D, T], bf16, kind="ExternalOutput")
    # group-3 projection partials (w0@x0 and w1@x1 halves, host-summed) so
    # half the tail projection can stream out during pair B
    d_out3 = nc.dram_tensor("out3", [2, D, 512], bf16, kind="ExternalOutput")

    with tile.TileContext(nc) as tc:
        with (
            tc.tile_pool(name="consts", bufs=1) as consts,
            tc.tile_pool(name="data", bufs=1) as data,
            tc.tile_pool(name="pt", bufs=8) as ptp,
            tc.tile_pool(name="small", bufs=6) as small,
            tc.tile_pool(name="psS", bufs=2, space="PSUM") as psS,
            tc.tile_pool(name="psO", bufs=1, space="PSUM") as psO,
        ):
            cosT = consts.tile([128, T], bf16)
            sinT = consts.tile([128, T], bf16)
            head = consts.tile([128, 2816], bf16)
            HQ0, HQ1, HRP, HMK, HKC, HCS, HSN = (0, 512, 1024, 1152, 1280,
                                                 1792, 2304)
            head2 = consts.tile([128, 2560], bf16)
            JQ0, JQ1, JKC, JCS, JSN = 0, 512, 1024, 1536, 2048
            ebias = consts.tile([128, 1], f32)
            nc.vector.memset(ebias[:], EBIAS)
            nc.sync.dma_start(head[:], d_head[:])

            qT = [data.tile([128, T], bf16, name=f"qT{i}", tag=f"qT{i}")
                  for i in range(2)]
            kT = data.tile([KVD, T], bf16, tag="kT")
            kThi = data.tile([128, T], bf16, tag="kThi")
            vaug = data.tile([128, NKB, HD + 1], bf16, tag="vaug")
            vaug8 = data.tile([128, NKB, 80], f8, tag="vaug8")
            w = [data.tile([128, D], bf16, name=f"w{i}", tag=f"w{i}")
                 for i in range(2)]
            xT = [data.tile([128, T], bf16, name=f"xT{i}", tag=f"xT{i}")
                  for i in range(2)]

            # tiny exp reading the locally-memset tile (no DMA dep) so
            # walrus's ACT_TABLE_LOAD runs immediately, before the ACT-ring
            # input DMAs occupy the queue
            dum = data.tile([128, 512], bf16, tag="dum")
            nc.vector.memset(dum[:], 1.0)
            # ones column for the tail's PE-side reciprocal broadcast
            onesc = data.tile([1, 64], f32, tag="onesc")
            nc.vector.memset(onesc[:], 1.0)
            dummy = small.tile([1, 8], bf16, tag="dummy")
            nc.scalar.activation(dummy[:], dum[0:1, 0:8], Exp, scale=0.125)

            # PE warmup: DMA-independent dense burst so HAM reaches full
            # clock early; 16 x N=512 cold matmuls (~7us) guarantee covering
            # a full free-running HAM activity window
            for _ in range(5):
                warm = psS.tile([128, 1024], f32, name="warm", tag="S")
                for j in range(2):
                    nc.tensor.matmul(warm[:, j * 512:(j + 1) * 512],
                                     dum[:, 0:128], dum[:],
                                     start=True, stop=True)

            # whole-tensor input DMAs. HWDGE issue is serial per ring
            # (~650ns each), so the rope-chunk-0 deps are split across BOTH
            # HWDGE rings (SP + ACT — ACT is idle until the first exp at
            # ~13us anyway); bulky contiguous loads ride the GpSimd SWDGE
            # ring. Ring order = first-use order.
            # Input DMA order = SDMA drain order = need order. After the
            # head slab: tails (cols 512:T) of q/k/cos/sin (rope chunk 1 at
            # ~12.5us needs their first third), then late consumers.
            tl = slice(1024, T)
            nc.sync.dma_start(head2[:], d_head2[:])
            nc.sync.dma_start(qT[0][:, tl], d_qT[0:128, tl])
            nc.sync.dma_start(kT[:, tl], d_kT[:, tl])
            nc.sync.dma_start(cosT[0:64, tl], d_cosT[:, tl])
            nc.sync.dma_start(sinT[0:64, tl], d_sinT[:, tl])
            nc.scalar.dma_start(cosT[64:128, tl], d_cosT[:, tl])
            nc.scalar.dma_start(sinT[64:128, tl], d_sinT[:, tl])
            nc.sync.dma_start(qT[1][:, tl], d_qT[128:256, tl])
            nc.gpsimd.dma_start(
                vaug[:].rearrange("p n m -> p (n m)"), d_vaug[:])
            nc.gpsimd.dma_start(
                vaug8[:].rearrange("p n m -> p (n m)"), d_vaug8[:])
            nc.gpsimd.dma_start(w[0][:], d_w[0:128, :])
            nc.gpsimd.dma_start(w[1][:], d_w[128:256, :])

            def rope_chunk_a(f, tq="oT0", tk="oT1"):
                # psum scratch borrows idle oT banks (oT2/3 during pair A
                # after the proj chunks; oT0/1 during prologue/pair B); the S
                # pool is never touched so the exp pipeline keeps its 2 slots.
                # Part a: qT[0] + kT (all the next pair A needs).
                sl = slice(f * 512, (f + 1) * 512)
                if f == 0:
                    qs = head[:, HQ0:HQ0 + 512]
                    ks = head[0:64, HKC:HKC + 512]
                    co, si = head[:, HCS:HCS + 512], head[:, HSN:HSN + 512]
                    co64 = head[0:64, HCS:HCS + 512]
                    si64 = head[0:64, HSN:HSN + 512]
                elif f == 1:
                    qs = head2[:, JQ0:JQ0 + 512]
                    ks = head2[0:64, JKC:JKC + 512]
                    co, si = head2[:, JCS:JCS + 512], head2[:, JSN:JSN + 512]
                    co64 = head2[0:64, JCS:JCS + 512]
                    si64 = head2[0:64, JSN:JSN + 512]
                else:
                    qs, ks = qT[0][:, sl], kT[:, sl]
                    co, si = cosT[:, sl], sinT[:, sl]
                    co64, si64 = cosT[:KVD, sl], sinT[:KVD, sl]
                rk = psO.tile([128, 512], f32, name="rk", tag=tk)
                nc.tensor.matmul(rk[:KVD, :], head[0:64, HRP:HRP + 64],
                                 ks, start=True, stop=True)
                rq0 = psO.tile([128, 512], f32, name="rq0", tag=tq)
                nc.tensor.matmul(rq0[:], head[:, HRP:HRP + 128], qs,
                                 start=True, stop=True)
                # independent muls first, psum-dependent ops after — the DVE
                # chain is the startup critical path
                nc.vector.tensor_mul(kT[:, sl], ks, co64)
                nc.vector.tensor_mul(qT[0][:, sl], qs, co)
                nc.vector.tensor_mul(rk[:KVD, :], rk[:KVD, :], si64)
                nc.vector.tensor_add(kT[:, sl], kT[:, sl], rk[:KVD, :])
                nc.vector.tensor_copy(kThi[64:128, sl], kT[:, sl])
                nc.vector.tensor_mul(rq0[:], rq0[:], si)
                nc.vector.tensor_add(qT[0][:, sl], qT[0][:, sl], rq0[:])

            def rope_chunk_b(f, t="oT0"):
                # Part b: qT[1]; emitted a few rounds after part a so the
                # rq1 matmul's WAR wait on the rq0 slot is already satisfied
                # (an unsatisfied wait here would stall the whole PE FIFO).
                sl = slice(f * 512, (f + 1) * 512)
                if f == 0:
                    qs = head[:, HQ1:HQ1 + 512]
                    co, si = head[:, HCS:HCS + 512], head[:, HSN:HSN + 512]
                elif f == 1:
                    qs = head2[:, JQ1:JQ1 + 512]
                    co, si = head2[:, JCS:JCS + 512], head2[:, JSN:JSN + 512]
                else:
                    qs = qT[1][:, sl]
                    co, si = cosT[:, sl], sinT[:, sl]
                rq1 = psO.tile([128, 512], f32, name="rq1", tag=t)
                nc.tensor.matmul(rq1[:], head[:, HRP:HRP + 128], qs,
                                 start=True, stop=True)
                nc.vector.tensor_mul(rq1[:], rq1[:], si)
                nc.vector.tensor_mul(qT[1][:, sl], qs, co)
                nc.vector.tensor_add(qT[1][:, sl], qT[1][:, sl], rq1[:])

            def proj_chunk(gq, j, slot):
                # out^T[n-chunk j, qlo:qlo+512] = sum_cc w[cc]^T @ xT[cc];
                # output DMAs alternate the SP and GpSimd rings
                qlo = gq * 512
                pr = psO.tile([128, 512], f32, name="pr", tag=f"oT{slot}")
                for cc in range(2):
                    nc.tensor.matmul(pr[:], w[cc][:, j * 128:(j + 1) * 128],
                                     xT[cc][:, qlo:qlo + 512],
                                     start=(cc == 0), stop=(cc == 1))
                st = ptp.tile([128, 512], bf16, name="st", tag="st")
                nc.vector.tensor_copy(st[:], pr[:])
                eng = nc.sync if j % 2 == 0 else nc.gpsimd
                eng.dma_start(d_outT[j * 128:(j + 1) * 128, qlo:qlo + 512],
                              st[:])

            def proj3_half(cc, j, slot, st_eng, dma_eng):
                # group-3 projection, single-cc half: streams to d_out3[cc]
                pr = psO.tile([128, 512], f32, name="pr3", tag=f"oT{slot}")
                nc.tensor.matmul(pr[:], w[cc][:, j * 128:(j + 1) * 128],
                                 xT[cc][:, 3 * 512:4 * 512],
                                 start=True, stop=True)
                st = ptp.tile([128, 512], bf16, name="st3", tag="st")
                st_eng(st[:], pr[:])
                dma_eng(d_out3[cc, j * 128:(j + 1) * 128, :], st[:])

            def attn_group(g):
                qlo = g * 512
                nkb = (qlo + 512) // QB
                kb0 = qlo // QB
                for pair in range(2):
                    qt = qT[pair]
                    xt = xT[pair]
                    oT = [psO.tile([HD + 1, 512], f32, name=f"oT{2*pair+j}",
                                   tag=f"oT{2*pair+j}") for j in range(2)]
                    PTd = None
                    # O matmuls are EMITTED 2 rounds late: in-order PE FIFO
                    # means an O placed right after its exp head-of-line
                    # blocks S(kb+1) behind exp(kb) — round time becomes
                    # S+exp+O serial instead of exp-paced. Two rounds of
                    # slack (the S-pool WAR already keeps PE <=2 rounds
                    # ahead) guarantees the O's inputs are long done.
                    pend = []

                    def flush_pend(upto_kb):
                        while pend and pend[0][0] <= upto_kb:
                            pend.pop(0)[1]()
                    for kb in range(nkb):
                        diag = kb >= kb0
                        cs = QB * (kb - kb0) if diag else 0
                        S2 = psS.tile([128, 1024], f32, name="S2", tag="S")
                        S2v = S2.rearrange("p (j n) -> p j n", j=2)
                        if g <= 1:
                            # keep-warm filler: early groups have too little
                            # PE work per round and HAM re-throttles the PE
                            # clock to 1.2 GHz without it
                            nc.tensor.matmul(S2[:, 0:128], dum[:, 0:128],
                                             dum[:, 0:128],
                                             start=True, stop=True)
                        for j in range(2):
                            qoff = j * 64
                            kTh = kT if j == 0 else kThi[64:128, :]
                            base = j * 512
                            nc.tensor.matmul(
                                S2[:, base + cs:base + 512],
                                kTh[:, kb * QB:(kb + 1) * QB],
                                qt[qoff:qoff + 64, qlo + cs:qlo + 512],
                                start=True, stop=True)
                        flush_pend(kb - 3)
                        if not diag:
                            # sub-diagonal rounds: exp straight to fp8 and
                            # accumulate O via DoubleRow (2 k-blocks per pass,
                            # 0.5 cyc/row) — diagonal strip stays bf16 so the
                            # dominant near-diagonal weights keep precision
                            if kb % 2 == 0:
                                PTd = ptp.tile([128, 2, 1024], f8,
                                               name="PTd", tag="PTd")
                            nc.scalar.activation(PTd[:, kb % 2, :], S2[:],
                                                 Exp, scale=0.125, bias=ebias[:])
                            if kb % 2 == 1:
                                def odr(kb=kb, PTd=PTd):
                                    for j in range(2):
                                        nc.tensor.matmul(
                                            oT[j][:, :],
                                            vaug8[:, kb - 1:kb + 1, 0:HD + 1],
                                            PTd[:, :, j * 512:(j + 1) * 512],
                                            start=(kb == 1), stop=False,
                                            perf_mode=DR,
                                            skip_group_check=True)
                                pend.append((kb, odr))
                        else:
                            PT = ptp.tile([128, 1024], bf16, name="PT",
                                          tag="PT")
                            if cs:
                                PTv = PT.rearrange("p (j n) -> p j n", j=2)
                                nc.scalar.activation(PTv[:, :, cs:],
                                                     S2v[:, :, cs:],
                                                     Exp, scale=0.125,
                                                     bias=ebias[:])
                            else:
                                nc.scalar.activation(PT[:], S2[:], Exp,
                                                     scale=0.125, bias=ebias[:])
                            # multiplicative causal mask on the diagonal
                            # 128-wide strip of each head's P^T (no psum
                            # has_written reliance — robust on HW)
                            for j in range(2):
                                nc.vector.tensor_mul(
                                    PT[:, j * 512 + cs:j * 512 + cs + QB],
                                    PT[:, j * 512 + cs:j * 512 + cs + QB],
                                    head[:, HMK:HMK + QB])

                            def odiag(kb=kb, cs=cs, PT=PT):
                                for j in range(2):
                                    nc.tensor.matmul(
                                        oT[j][:, cs:], vaug[:, kb, :],
                                        PT[:, j * 512 + cs:(j + 1) * 512],
                                        start=(kb == 0), stop=(kb == nkb - 1),
                                        skip_group_check=True)
                            pend.append((kb, odiag))
                        # deferred work hooks. For g1/g2 the projection
                        # burst is split across BOTH pairs (4 chunks each) —
                        # 8 chunks in pair A's first rounds starve ACT for
                        # ~4.6us; pair B's kb2/kb4 use the freed oT0/1 slots
                        if pair == 0 and 1 <= g <= 2 and 1 <= kb <= 4:
                            proj_chunk(g - 1, kb - 1, 2 + (kb % 2))
                        if pair == 0 and g == 3 and 1 <= kb <= 8:
                            proj_chunk(g - 1, kb - 1, 2 + (kb % 2))
                        if pair == 1 and 1 <= g <= 2 and kb in (2, 4):
                            for i in range(2):
                                proj_chunk(g - 1, 4 + (kb // 2 - 1) * 2 + i, i)
                        if pair == 0 and 1 <= g <= 2 and kb == 6:
                            # rope the next chunk early (pair A, borrowing
                            # the proj banks) so the DVE backlog of pair B
                            # can't delay the next group's q/k readiness
                            rope_chunk_a(g + 1, tq="oT2", tk="oT3")
                        if pair == 0 and g == 0 and kb == 0:
                            # q1 chunk 0 rides the head slab (lands ~10us);
                            # roping it here keeps the prologue FIFOs clear
                            rope_chunk_b(0, t="oT2")
                        if pair == 0 and g == 0 and kb == 1:
                            rope_chunk_a(1, tq="oT2", tk="oT3")
                        if pair == 0 and g == 0 and kb == 3:
                            rope_chunk_b(1, t="oT2")
                        if pair == 1 and kb == min(3, nkb - 2) and 1 <= g <= 2:
                            rope_chunk_b(g + 1)
                        if pair == 1 and g == 3 and 4 <= kb <= 11:
                            # stream group-3's w0@x0 projection half during
                            # pair B (oT0/oT1 banks are free: no rope here)
                            j3 = kb - 4
                            proj3_half(0, j3, j3 % 2,
                                       nc.vector.tensor_copy,
                                       nc.gpsimd.dma_start)
                    flush_pend(nkb)
                    # normalize the pair; reciprocal_approx_fast needs SBUF
                    # input (PSUM source diverges on HW), so stage the
                    # denominator row first. In the tail (last pair) the
                    # broadcast runs on the idle PE (ones-column matmul into
                    # free psum) instead of the ~1us gpsimd broadcast.
                    last = (g == NG - 1 and pair == 1)
                    for j in range(2):
                        qoff = j * 64
                        den = small.tile([1, 512], f32, tag="den")
                        if last:
                            # tail only: den staging on the now-idle ACT
                            # shortens the end-of-kernel serial chain (~1us)
                            nc.scalar.copy(den[:], oT[j][HD:HD + 1, :])
                        else:
                            nc.vector.tensor_copy(den[:], oT[j][HD:HD + 1, :])
                        rec = small.tile([1, 512], f32, tag="rec")
                        nc.vector.reciprocal_approx_fast(rec[:], den[:])
                        bcs = small.tile([64, 512], f32, tag="bcs")
                        nc.gpsimd.partition_broadcast(bcs[:], rec[:])
                        nc.vector.tensor_mul(xt[qoff:qoff + 64, qlo:qlo + 512],
                                             oT[j][:HD, :], bcs[:])

            rope_chunk_a(0)
            for g in range(NG):
                attn_group(g)
            # tail fillers: bridge the pair-B normalize window so the PE
            # clock stays at 8/8 for the final projection matmuls
            for _ in range(2):
                warm2 = psS.tile([128, 1024], f32, name="warm2", tag="S")
                for j in range(2):
                    nc.tensor.matmul(warm2[:, j * 512:(j + 1) * 512],
                                     dum[:, 0:128], dum[:],
                                     start=True, stop=True)
            # tail: group-3's w1@x1 half, spread across all three DMA rings
            for j in range(D // 128):
                st_eng = (nc.vector.tensor_copy if j % 2 == 0
                          else nc.scalar.copy)
                dma_eng = [nc.sync.dma_start, nc.scalar.dma_start,
                           nc.gpsimd.dma_start][j % 3]
                proj3_half(1, j, j % 4, st_eng, dma_eng)

    nc.finalize()
    return nc


def _get_nc():
    if "nc" not in _cache:
        _cache["nc"] = _build()
    return _cache["nc"]


def _in_maps(q, k, v, w_out):
    import ml_dtypes
    bf = ml_dtypes.bfloat16
    f8 = ml_dtypes.float8_e4m3
    cosT, sinT, rotP, mask01 = _tables()
    ones = np.ones((T, 1), np.float32)
    maps = []
    for c in range(NCORES):
        b, kv = divmod(c, NKV)
        va = np.ascontiguousarray(
            np.concatenate([v[b, :, kv * KVD:(kv + 1) * KVD], ones], 1))
        # device SBUF layout [p, kb, m]: row p holds k-position kb*128+p
        va_p = va.reshape(NKB, 128, HD + 1).transpose(1, 0, 2)
        va8_p = np.pad(va_p, ((0, 0), (0, 0), (0, 80 - (HD + 1))))
        qTc = np.ascontiguousarray(q[b, :, kv * CD:(kv + 1) * CD].T)
        kTc = np.ascontiguousarray(k[b, :, kv * KVD:(kv + 1) * KVD].T)
        head = np.zeros((128, 2816), np.float32)
        head[:, 0:512] = qTc[0:128, 0:512]
        head[:, 512:1024] = qTc[128:256, 0:512]
        head[:, 1024:1152] = rotP
        head[:, 1152:1280] = mask01
        head[0:64, 1280:1792] = kTc[:, 0:512]
        head[:, 1792:2304] = np.vstack([cosT[:, 0:512]] * 2)
        head[:, 2304:2816] = np.vstack([sinT[:, 0:512]] * 2)
        head2 = np.zeros((128, 2560), np.float32)
        head2[:, 0:512] = qTc[0:128, 512:1024]
        head2[:, 512:1024] = qTc[128:256, 512:1024]
        head2[0:64, 1024:1536] = kTc[:, 512:1024]
        head2[:, 1536:2048] = np.vstack([cosT[:, 512:1024]] * 2)
        head2[:, 2048:2560] = np.vstack([sinT[:, 512:1024]] * 2)
        maps.append({
            "qT": qTc.astype(bf),
            "kT": kTc.astype(bf),
            "head": head.astype(bf),
            "head2": head2.astype(bf),
            "vaug": np.ascontiguousarray(va_p.reshape(128, -1)).astype(bf),
            "vaug8": np.ascontiguousarray(
                np.clip(va8_p, -240, 240).reshape(128, -1)).astype(f8),
            "w": np.ascontiguousarray(w_out[kv * CD:(kv + 1) * CD, :]).astype(bf),
            "cosT": cosT.astype(bf), "sinT": sinT.astype(bf),
        })
    return maps


def _run(q, k, v, w_out, trace=False):
    from concourse.bass_utils import run_bass_kernel_spmd

    nc = _get_nc()
    res = run_bass_kernel_spmd(nc, _in_maps(q, k, v, w_out),
                               core_ids=list(range(NCORES)), trace=trace)
    out = np.zeros((B, T, D), np.float32)
    for c in range(NCORES):
        ot = res.results[c]["outT"].T.astype(np.float32)
        ot[3 * 512:] = 0.0   # q-cols 1536:2048 come from out3 instead
        out[c // NKV] += ot
        o3 = res.results[c]["out3"].astype(np.float32)
        out[c // NKV][3 * 512:] += (o3[0] + o3[1]).T
    return out, res


def kernel(q, k, v, w_out):
    out, _ = _run(np.asarray(q), np.asarray(k), np.asarray(v),
                  np.asarray(w_out))
    return out

